# revision 1
# baseline (speedup 1.0000x reference)
"""DeMBR multi-behavior LightGCN kernel for Trainium2 (8 NeuronCores).

Strategy (per dense behavior, each [N,N] relation matrix R):
  - Host pre-casts R to bf16. Row-shard across 8 cores (512 users each).
  - Each core loads its shard twice from HBM, both as plain HWDGE DMAs on
    separate FIFOs: natural layout (streamed in 8 column chunks) and
    transposed layout (DMA-xbar transpose-load straight from DRAM).
  - All propagation products are PE matmuls with the big matrix as the
    moving operand (512-wide free dim):
      C2:    u1_un.T (+deg_u via ones column)  = [i0|1].T @ R^T-shard
      C1+C4: z.T / w.T packed                  = [u0|u0+u1].T @ R-shard
      C3:    u2_un.T                           = i1.T @ R^T-shard
  - One [64, 4096] fp32 AllReduce per behavior combines z = R^T u0 across
    cores (needed on-device for the layer-2 user side). The item-side
    output i_acc = (z + R^T u1) / (2 deg_i) is assembled on the host from
    the AllReduced z plus per-core w = R^T(u0+u1) partials.
  - deg_i (a column-sum of the input matrix) is computed on the host in one
    pass; its reciprocal is fed to the device for the i1 normalization.
    deg_u comes free as the ones-column of the C2 matmul.
  - All-ones matrices (the virtual-behavior M's at init) are detected on
    the host and computed analytically (ones @ X is a broadcast column sum).

kernel(**inputs) takes the full unsharded inputs and returns [14, 4096, 64].
"""

import os
import numpy as np
import ml_dtypes

EPS = 1e-8
N, D = 4096, 64
P = 128
NCORES = 8
ULOC = N // NCORES          # 512 users per core
NU = ULOC // P              # 4 user chunks
NI = N // P                 # 32 item chunks
CH = 512                    # moving free-dim chunk
NCH = N // CH               # 8 chunks for the user-side contractions

_BF16 = ml_dtypes.bfloat16


# --------------------------------------------------------------------------
# device program
# --------------------------------------------------------------------------

def build_program(nb):
    """Build + bacc-compile the SPMD program for `nb` dense behaviors."""
    import concourse.bass as bass  # noqa: F401  (registers types)
    import concourse.mybir as mybir
    import concourse.tile as tile
    from concourse import bacc
    from concourse.masks import make_identity

    f32, bf16 = mybir.dt.float32, mybir.dt.bfloat16
    ALU = mybir.AluOpType
    CPY = mybir.ActivationFunctionType.Copy

    nc = bacc.Bacc("TRN2", target_bir_lowering=False, debug=False,
                   num_devices=NCORES)

    R_in = [nc.dram_tensor(f"R{b}", [ULOC, N], bf16, kind="ExternalInput")
            for b in range(nb)]
    Rt_in = [nc.dram_tensor(f"Rt{b}", [N, ULOC], bf16, kind="ExternalInput")
             for b in range(nb)]
    ri_in = [nc.dram_tensor(f"ri{b}", [P, NI], f32, kind="ExternalInput")
             for b in range(nb)]
    i0s_in = nc.dram_tensor("i0s", [P, NI, D + 1], bf16, kind="ExternalInput")
    u0s_in = nc.dram_tensor("u0s", [P, NU, D], bf16, kind="ExternalInput")
    uacc_out = [nc.dram_tensor(f"uacc{b}", [P, NU, D], f32, kind="ExternalOutput")
                for b in range(nb)]
    w_out = [nc.dram_tensor(f"wT{b}", [D, N], f32, kind="ExternalOutput")
             for b in range(nb)]

    rg = [list(range(NCORES))]

    with tile.TileContext(nc) as tc:
        with (
            tc.tile_pool(name="big", bufs=4) as pbig,
            tc.tile_pool(name="chunk", bufs=4) as pchunk,
            tc.tile_pool(name="small", bufs=2) as psm,
            tc.tile_pool(name="one", bufs=1) as pone,
            tc.tile_pool(name="uat", bufs=4) as pu,
            tc.tile_pool(name="mm", bufs=2, space="PSUM") as pmm,
            tc.tile_pool(name="mm14", bufs=3, space="PSUM") as pmm14,
            tc.tile_pool(name="tr", bufs=1, space="PSUM") as ptr,
            tc.tile_pool(name="dram", bufs=4, space="DRAM") as pdr,
        ):
            ident = pone.tile([P, P], f32)
            make_identity(nc, ident[:])
            i0s = pone.tile([P, NI, D + 1], bf16)
            nc.sync.dma_start(out=i0s[:], in_=i0s_in[:])
            u0s = pone.tile([P, NU, D], bf16)
            nc.sync.dma_start(out=u0s[:], in_=u0s_in[:])

            state = {}
            at_tiles = {}
            uacc_tiles = {}

            def load_at(b):
                # ---- R^T copy: plain strided load of the host-pretransposed
                #      shard (scalar-engine HWDGE FIFO), split in 4 so the
                #      first C2 matmuls can start early
                At = pbig.tile([P, NI, ULOC], bf16, tag="At", name=f"At{b}")
                q = NI // 4
                src = Rt_in[b].ap().rearrange("(ic p) u -> p ic u", p=P)
                for g in range(4):
                    nc.scalar.dma_start(out=At[:, g * q:(g + 1) * q, :],
                                        in_=src[:, g * q:(g + 1) * q, :])
                at_tiles[b] = At

            def front(b):
                At = at_tiles.pop(b)
                ris = psm.tile([P, NI], f32, tag="ris", name=f"ris{b}")
                nc.sync.dma_start(out=ris[:], in_=ri_in[b].ap())

                # ---- C2: psum [65, 512] = [i0|1].T @ R^T  (accum over items)
                P2 = pmm.tile([D + 1, CH], f32, tag="PC", name=f"P2_{b}")
                for ic in range(NI):
                    nc.tensor.matmul(P2[:], i0s[:, ic, :], At[:, ic, :],
                                     start=(ic == 0), stop=(ic == NI - 1))
                S2 = psm.tile([D + 1, CH], f32, tag="S2", name=f"S2_{b}")
                nc.vector.tensor_copy(out=S2[:], in_=P2[:])
                PT2 = ptr.tile([P, NU, D + 1], f32, tag="PT2", name=f"PT2_{b}")
                for uc in range(NU):
                    nc.tensor.transpose(PT2[:, uc, :],
                                        S2[:, uc * P:(uc + 1) * P],
                                        ident[0:D + 1, 0:D + 1])
                rut = psm.tile([P, NU, 1], f32, tag="rut", name=f"rut{b}")
                nc.vector.tensor_scalar_add(out=rut[:], in0=PT2[:, :, D:D + 1],
                                            scalar1=EPS)
                ru = psm.tile([P, NU, 1], f32, tag="ru", name=f"ru{b}")
                nc.vector.reciprocal(out=ru[:], in_=rut[:])
                u1f = psm.tile([P, NU, D], f32, tag="u1f", name=f"u1f{b}")
                for uc in range(NU):
                    nc.vector.tensor_scalar_mul(out=u1f[:, uc, :],
                                                in0=PT2[:, uc, 0:D],
                                                scalar1=ru[:, uc, :])
                L = psm.tile([P, NU, 2 * D], bf16, tag="L", name=f"L{b}")
                nc.vector.tensor_copy(out=L[:, :, 0:D], in_=u0s[:])
                u1b = psm.tile([P, NU, D], bf16, tag="u1b", name=f"u1b{b}")
                nc.vector.tensor_copy(out=u1b[:], in_=u1f[:])
                nc.vector.tensor_add(out=L[:, :, D:2 * D], in0=u0s[:], in1=u1b[:])

                # ---- C1+C4 packed: [u0 | u0+u1].T @ R-shard -> z.T | w.T
                #      R-shard streamed from DRAM in [128, 4, 512] chunks
                zT = pone.tile([D, N], bf16, tag="zT", name=f"zT{b}")
                wT = pone.tile([D, N], f32, tag="wT", name=f"wT{b}")
                for n in range(NCH):
                    Ac = pchunk.tile([P, NU, CH], bf16, tag="Ac",
                                     name=f"Ac{b}_{n}")
                    nc.sync.dma_start(
                        out=Ac[:],
                        in_=R_in[b].ap().rearrange(
                            "(uc p) n -> p uc n", p=P)[:, :,
                                                       n * CH:(n + 1) * CH])
                    P14 = pmm14.tile([P, CH], f32, tag="P14",
                                     name=f"P14_{b}_{n}")
                    for uc in range(NU):
                        nc.tensor.matmul(P14[:], L[:, uc, :], Ac[:, uc, :],
                                         start=(uc == 0), stop=(uc == NU - 1))
                    nc.vector.tensor_copy(out=zT[:, n * CH:(n + 1) * CH],
                                          in_=P14[0:D, :])
                    nc.vector.tensor_copy(out=wT[:, n * CH:(n + 1) * CH],
                                          in_=P14[D:2 * D, :])

                # ---- AllReduce z (bf16: only feeds the layer-2 user side).
                #      gpsimd runs nothing but the collective doorbell/wait
                #      pairs, so the chain has no queue interleaving
                z_in = pdr.tile([D, N], bf16, tag="z_in", name=f"z_in{b}")
                nc.sync.dma_start(out=z_in[:], in_=zT[:])
                nc.sync.dma_start(out=w_out[b].ap(), in_=wT[:])
                z_out = pdr.tile([D, N], bf16, tag="z_out",
                                 name=f"z_out{b}", addr_space="Shared")
                nc.gpsimd.collective_compute(
                    "AllReduce", ALU.add, replica_groups=rg,
                    ins=[z_in.opt()], outs=[z_out.opt()])
                state[b] = (At, ris, ru, u1f, z_out)

            def back(b):
                At, ris, ru, u1f, z_out = state.pop(b)

                # ---- i1 = z * ri in natural layout (xbar transpose +
                #      in-place per-item scale on DVE)
                zs = pone.tile([D, N], bf16, tag="zs", name=f"zs{b}")
                nc.scalar.dma_start(out=zs[:], in_=z_out[:])
                i1b = pone.tile([P, NI, D], bf16, tag="i1b", name=f"i1b{b}")
                nc.scalar.dma_start_transpose(out=i1b[:], in_=zs[:])
                for ic in range(NI):
                    nc.vector.tensor_scalar_mul(out=i1b[:, ic, :],
                                                in0=i1b[:, ic, :],
                                                scalar1=ris[:, ic:ic + 1])

                # ---- C3: u2_un.T = i1.T @ R^T
                P3 = pmm.tile([D, CH], f32, tag="PC", name=f"P3_{b}")
                for ic in range(NI):
                    nc.tensor.matmul(P3[:], i1b[:, ic, :], At[:, ic, :],
                                     start=(ic == 0), stop=(ic == NI - 1))
                S3 = psm.tile([D, CH], f32, tag="S3", name=f"S3_{b}")
                nc.vector.tensor_copy(out=S3[:], in_=P3[:])
                PT3 = ptr.tile([P, NU, D], f32, tag="PT3", name=f"PT3_{b}")
                for uc in range(NU):
                    nc.tensor.transpose(PT3[:, uc, :],
                                        S3[:, uc * P:(uc + 1) * P],
                                        ident[0:D, 0:D])
                uacc = pu.tile([P, NU, D], f32, tag="uacc", name=f"uacc{b}")
                for uc in range(NU):
                    nc.vector.scalar_tensor_tensor(
                        out=uacc[:, uc, :], in0=PT3[:, uc, :],
                        scalar=ru[:, uc, :], in1=u1f[:, uc, :],
                        op0=ALU.mult, op1=ALU.add)

                uacc_tiles[b] = uacc

            # all At loads prefetch first; all fronts run, with one grouped
            # AllReduce per behavior pair; backs (post-AR) come last so no
            # engine queue ever stalls on a collective
            for b in range(min(nb, 4)):
                load_at(b)
            fe = be = 0
            while be < nb:
                if fe < nb and fe - be < 4:
                    if fe >= 4:
                        load_at(fe)
                    front(fe)
                    fe += 1
                else:
                    back(be)
                    be += 1
            # all user-side outputs at the very end: keeps the DMA FIFOs
            # free of post-AllReduce waits while fronts are streaming
            for b in range(nb):
                nc.sync.dma_start(out=uacc_out[b].ap(), in_=uacc_tiles[b][:])

    nc.compile()
    return nc


# --------------------------------------------------------------------------
# host-side helpers
# --------------------------------------------------------------------------

def _swz_items(x):
    """[4096, C] -> [128, 32, C] with item = ic*128 + p."""
    return np.ascontiguousarray(x.reshape(NI, P, x.shape[1]).transpose(1, 0, 2))


def _swz_users(x):
    """[512, C] -> [128, 4, C] with user = uc*128 + p."""
    return np.ascontiguousarray(x.reshape(NU, P, x.shape[1]).transpose(1, 0, 2))


def prep_in_maps(dense_mats, u0, i0):
    """dense_mats: list of (R_bf16 [N,N], ri_nat [128, 32] f32)."""
    i0_aug = np.concatenate(
        [i0.astype(_BF16), np.ones((N, 1), _BF16)], axis=1)
    i0s = _swz_items(i0_aug)
    in_maps = []
    for k in range(NCORES):
        m = {"i0s": i0s,
             "u0s": _swz_users(u0[k * ULOC:(k + 1) * ULOC].astype(_BF16))}
        for b, (Rb, ri_nat) in enumerate(dense_mats):
            m[f"R{b}"] = np.ascontiguousarray(Rb[k * ULOC:(k + 1) * ULOC, :])
            m[f"Rt{b}"] = _per_core_rt(Rb, k)
            m[f"ri{b}"] = ri_nat
        in_maps.append(m)
    return in_maps


def host_prep_behavior(R):
    """Cast to bf16 + compute item-degree reciprocal (natural layout)."""
    Rb = R.astype(_BF16)
    deg = R.sum(axis=0, dtype=np.float64)
    ri_vec = (1.0 / (deg + EPS)).astype(np.float32)
    ri_nat = np.ascontiguousarray(ri_vec.reshape(NI, P).T)
    return Rb, ri_nat, deg.astype(np.float32)


def _per_core_rt(Rb, k):
    """Contiguous [N, ULOC] transposed shard for core k."""
    return np.ascontiguousarray(Rb[k * ULOC:(k + 1) * ULOC, :].T)


def assemble_dense(results, degs, nb):
    """Per-behavior (u_acc [N,D], i_acc [N,D]) from per-core outputs."""
    out = []
    for b in range(nb):
        u = np.concatenate(
            [results[k][f"uacc{b}"].transpose(1, 0, 2).reshape(ULOC, D)
             for k in range(NCORES)], axis=0) * np.float32(0.5)
        w = np.sum([results[k][f"wT{b}"] for k in range(NCORES)], axis=0,
                   dtype=np.float32)
        i_acc = (w * np.float32(0.5)
                 / (degs[b] + np.float32(EPS))[None, :]).T
        out.append((np.ascontiguousarray(u, dtype=np.float32),
                    np.ascontiguousarray(i_acc, dtype=np.float32)))
    return out


def ones_behavior(u0, i0):
    """Analytic LightGCN-2-layer outputs when R is all-ones [N, N]."""
    s_i = i0.astype(np.float64).sum(axis=0)
    s_u = u0.astype(np.float64).sum(axis=0)
    d = N + EPS
    u_row = (s_i / d + s_u * N / (d * d)) * 0.5
    i_row = (s_u / d + s_i * N / (d * d)) * 0.5
    u = np.broadcast_to(u_row.astype(np.float32), (N, D)).copy()
    it = np.broadcast_to(i_row.astype(np.float32), (N, D)).copy()
    return u, it


# --------------------------------------------------------------------------
# cached device runner (compile once per behavior-count, run many)
# --------------------------------------------------------------------------

_RUNNERS = {}


class _Runner:
    def __init__(self, nb):
        self.nb = nb
        self.nc = build_program(nb)
        self._jitted = None
        self._meta = None

    def _prep_jit(self):
        import jax
        import numpy as _np
        from jax.sharding import Mesh, PartitionSpec
        from jax.experimental.shard_map import shard_map
        from concourse import bass2jax
        from concourse.bass2jax import _bass_exec_p, partition_id_tensor
        import concourse.mybir as mybir

        bass2jax.install_neuronx_cc_hook()
        nc = self.nc
        partition_name = (nc.partition_id_tensor.name
                          if nc.partition_id_tensor else None)
        in_names, out_names, out_avals, zero_shapes = [], [], [], []
        for alloc in nc.m.functions[0].allocations:
            if not isinstance(alloc, mybir.MemoryLocationSet):
                continue
            name = alloc.memorylocations[0].name
            if alloc.kind == "ExternalInput":
                if name != partition_name:
                    in_names.append(name)
            elif alloc.kind == "ExternalOutput":
                shape = tuple(alloc.tensor_shape)
                dtype = mybir.dt.np(alloc.dtype)
                out_names.append(name)
                out_avals.append(jax.core.ShapedArray(shape, dtype))
                zero_shapes.append((shape, dtype))
        n_params = len(in_names)
        full_in_names = list(in_names) + list(out_names)
        if partition_name is not None:
            full_in_names.append(partition_name)

        def _body(*args):
            operands = list(args)
            if partition_name is not None:
                operands.append(partition_id_tensor())
            outs = _bass_exec_p.bind(
                *operands,
                out_avals=tuple(out_avals),
                in_names=tuple(full_in_names),
                out_names=tuple(out_names),
                lowering_input_output_aliases=(),
                sim_require_finite=True,
                sim_require_nnan=True,
                nc=nc,
            )
            return tuple(outs)

        devices = jax.devices()[:NCORES]
        mesh = Mesh(_np.asarray(devices), ("core",))
        n_outs = len(out_names)
        in_specs = (PartitionSpec("core"),) * (n_params + n_outs)
        out_specs = (PartitionSpec("core"),) * n_outs
        donate = tuple(range(n_params, n_params + n_outs))
        self._jitted = jax.jit(
            shard_map(_body, mesh=mesh, in_specs=in_specs,
                      out_specs=out_specs, check_rep=False),
            donate_argnums=donate, keep_unused=True)
        self._meta = (in_names, out_names, out_avals, zero_shapes, n_params)

    def run(self, in_maps):
        if self._jitted is None:
            self._prep_jit()
        import numpy as _np
        in_names, out_names, out_avals, zero_shapes, n_params = self._meta
        concat_in = [
            _np.concatenate([_np.asarray(in_maps[c][nm]) for c in range(NCORES)],
                            axis=0)
            for nm in in_names]
        concat_zeros = [_np.zeros((NCORES * s[0], *s[1:]), dt)
                        for (s, dt) in zero_shapes]
        out_arrs = self._jitted(*concat_in, *concat_zeros)
        results = []
        for c in range(NCORES):
            results.append({
                nm: _np.asarray(out_arrs[i]).reshape(
                    NCORES, *out_avals[i].shape)[c]
                for i, nm in enumerate(out_names)})
        return results

    def run_traced(self, in_maps, tmpdir=None):
        """Run through run_bass_kernel_spmd with NTFF tracing (recompiles)."""
        _install_trace_shims()
        from concourse.bass_utils import run_bass_kernel_spmd
        return run_bass_kernel_spmd(self.nc, in_maps,
                                    core_ids=list(range(NCORES)),
                                    trace=True, tmpdir=tmpdir)


def _install_trace_shims():
    """This image's antenv lacks axon_hooks (the NTFF-hook registry) and has
    no artifact bucket; recreate the hook from the boot recipe and make
    artifact upload a local no-op."""
    import sys, types, importlib.util

    if "antenv.axon_hooks" not in sys.modules:
        mod = types.ModuleType("antenv.axon_hooks")
        mod._hook = None

        def set_axon_ntff_profile_hook(h):
            mod._hook = h

        def get_axon_ntff_profile_hook():
            return mod._hook

        mod.set_axon_ntff_profile_hook = set_axon_ntff_profile_hook
        mod.get_axon_ntff_profile_hook = get_axon_ntff_profile_hook
        import antenv
        sys.modules["antenv.axon_hooks"] = mod
        antenv.axon_hooks = mod

        spec = importlib.util.spec_from_file_location(
            "trn_boot_shim", "/root/.axon_site/trn_agent_boot/trn_boot.py")
        boot = importlib.util.module_from_spec(spec)
        spec.loader.exec_module(boot)
        hook = boot._ntff_profile_via_ctypes("/opt/axon/libaxon_pjrt.so")
        mod._hook = hook

    import concourse.bass_utils as bu
    if not getattr(bu.upload_artifacts, "_is_local_shim", False):
        def _local_upload(tmpdir):
            return tmpdir
        _local_upload._is_local_shim = True
        bu.upload_artifacts = _local_upload


def get_runner(nb):
    if nb not in _RUNNERS:
        _RUNNERS[nb] = _Runner(nb)
    return _RUNNERS[nb]


# --------------------------------------------------------------------------
# entry point
# --------------------------------------------------------------------------

def _is_ones(a):
    return a[0, 0] == 1.0 and bool(np.all(a == np.float32(1.0)))


def kernel(**inputs):
    inputs = {k: np.asarray(v) for k, v in inputs.items()}
    u0 = np.ascontiguousarray(inputs["user_embedding"], dtype=np.float32)
    i0 = np.ascontiguousarray(inputs["item_embedding"], dtype=np.float32)

    real_names = ["R_click", "R_fav", "R_cart", "R_buy"]
    virt_names = [("M_click", "add_click"), ("M_fav", "add_fav"),
                  ("M_cart", "add_cart")]
    mats = [np.asarray(inputs[n], dtype=np.float32) for n in real_names]
    mats += [np.asarray(inputs[m], dtype=np.float32) for m, _ in virt_names]

    dense_idx = [j for j, a in enumerate(mats) if not _is_ones(a)]
    per_behavior = [None] * 7

    if dense_idx:
        nb = len(dense_idx)
        runner = get_runner(nb)
        prepped = [host_prep_behavior(mats[j]) for j in dense_idx]
        in_maps = prep_in_maps([(p[0], p[1]) for p in prepped], u0, i0)
        results = runner.run(in_maps)
        dense = assemble_dense(results, [p[2] for p in prepped], nb)
        for pos, j in enumerate(dense_idx):
            per_behavior[j] = dense[pos]

    ones_cache = None
    for j, a in enumerate(mats):
        if per_behavior[j] is None:
            if ones_cache is None:
                ones_cache = ones_behavior(u0, i0)
            per_behavior[j] = ones_cache

    ur = [per_behavior[j][0] for j in range(4)]
    ir = [per_behavior[j][1] for j in range(4)]
    uv = [per_behavior[4 + j][0] + np.asarray(inputs[virt_names[j][1]],
                                              dtype=np.float32)
          for j in range(3)]
    iv = [per_behavior[4 + j][1] for j in range(3)]

    out = np.concatenate(
        [np.stack(ur), np.stack(ir), np.stack(uv), np.stack(iv)], axis=0)
    return np.ascontiguousarray(out, dtype=np.float32)



# revision 4
# speedup vs baseline: 1.6732x; 1.6732x over previous
"""DeMBR multi-behavior LightGCN kernel for Trainium2 (8 NeuronCores) — v2.

Strategy (per dense behavior, each [N,N] relation matrix R, fp8-e4m3):
  Core k holds TWO different 1/8 shards of R, host pre-swizzled into flat
  SBUF layouts so every bulk DMA is 128 partitions x >=8KB contiguous:
    A = row-shard,    transposed layout [128p=item, 32ic, 512u]
        (contract over items -> all R@x products for the local 512 users)
    B = column-shard, natural layout    [128p=user, 32uc, 512i]
        (contract over users -> all R^T@y products for the local 512 items,
         FULLY reduced on-device; no AllReduce, no partial-sum outputs)

  All matmuls are fp8 DoubleRow (K=256 per MM, 2 MACs/cell/cycle):
    P1: u1_raw.T [64,512]  = i0.T @ A                     (layer-1 user side)
    P2: [i1_raw | 64*i2_raw].T [128,512] = [u0 | u1q].T @ B   (packed i-side)
    P3: 64*u2_raw.T [64,512] = i1q.T @ A                  (layer-2 user side)
  where u1q = fp8(64*u1_raw/deg_u) and i1q = fp8(64*i1_raw/deg_i) are
  exchanged between the two layers with one small AllGather each
  ([128,4,64] fp8 = 32KB per core -> [1024,4,64]); the x64 prescale keeps
  the tiny normalized embeddings in fp8-normal range and is divided back
  out on-device (scalar_tensor_tensor) before the raw sums leave the chip.

  Device outputs stay in [64, 512] transposed layout; the host applies the
  0.5/deg scaling and the transpose during assembly.  deg_u/deg_i are
  host-computed column/row sums of the ORIGINAL fp32 matrix.

  All-ones matrices (the virtual-behavior M's at init) are detected on the
  host and computed analytically.

kernel(**inputs) takes the full unsharded inputs and returns [14, 4096, 64].
"""

import os
import numpy as np
import ml_dtypes

EPS = 1e-8
N, D = 4096, 64
P = 128
NCORES = 8
ULOC = N // NCORES          # 512 local users/items per core
NC32 = N // P               # 32 chunks of 128 covering the contraction dim
NG = NC32 // 2              # 16 DoubleRow k-groups (pairs of chunks)
C4 = ULOC // P              # 4 local 128-chunks
SC = 64.0                   # power-of-2 prescale for fp8 contributions

_FP8 = ml_dtypes.float8_e4m3   # == mybir.dt.np(mybir.dt.float8e4)


# --------------------------------------------------------------------------
# device program
# --------------------------------------------------------------------------

def build_program(nb):
    """Build + bacc-compile the SPMD program for `nb` dense behaviors."""
    import concourse.bass as bass  # noqa: F401  (registers types)
    import concourse.mybir as mybir
    import concourse.tile as tile
    from concourse import bacc
    from concourse.masks import make_identity

    f32, fp8 = mybir.dt.float32, mybir.dt.float8e4
    ALU = mybir.AluOpType
    DR = mybir.MatmulPerfMode.DoubleRow

    nc = bacc.Bacc("TRN2", target_bir_lowering=False, debug=False,
                   num_devices=NCORES)

    A_in = [nc.dram_tensor(f"A{b}", [P, NC32, ULOC], fp8, kind="ExternalInput")
            for b in range(nb)]
    B_in = [nc.dram_tensor(f"B{b}", [P, NC32, ULOC], fp8, kind="ExternalInput")
            for b in range(nb)]
    ru_in = [nc.dram_tensor(f"ru{b}", [P, C4, 1], f32, kind="ExternalInput")
             for b in range(nb)]
    ri_in = [nc.dram_tensor(f"ri{b}", [P, C4, 1], f32, kind="ExternalInput")
             for b in range(nb)]
    i0s_in = nc.dram_tensor("i0s", [P, NC32, D], fp8, kind="ExternalInput")
    u0s_in = nc.dram_tensor("u0s", [P, NC32, D], fp8, kind="ExternalInput")
    uT_out = [nc.dram_tensor(f"uT{b}", [D, ULOC], f32, kind="ExternalOutput")
              for b in range(nb)]
    iT_out = [nc.dram_tensor(f"iT{b}", [D, ULOC], f32, kind="ExternalOutput")
              for b in range(nb)]

    rg = [list(range(NCORES))]
    nsh = min(nb, 4)

    with tile.TileContext(nc) as tc:
        with (
            tc.tile_pool(name="shA", bufs=nsh) as pshA,
            tc.tile_pool(name="shB", bufs=nsh) as pshB,
            tc.tile_pool(name="one", bufs=1) as pone,
            tc.tile_pool(name="deg", bufs=nb) as pdeg,
            tc.tile_pool(name="sm", bufs=4) as psm,
            tc.tile_pool(name="s1", bufs=min(nb, 4)) as pS1,
            tc.tile_pool(name="fb", bufs=4) as pfb,
            tc.tile_pool(name="lt", bufs=2) as pL,
            tc.tile_pool(name="out", bufs=2) as pout,
            tc.tile_pool(name="ps13", bufs=2, space="PSUM") as pps13,
            tc.tile_pool(name="ps2", bufs=2, space="PSUM") as pps2,
            tc.tile_pool(name="ptr", bufs=2, space="PSUM") as ptr,
            tc.tile_pool(name="dram", bufs=4, space="DRAM") as pdr,
        ):
            ident = pone.tile([P, P], f32)
            make_identity(nc, ident[:])
            i0s = pone.tile([P, NC32, D], fp8, tag="i0s")
            nc.scalar.dma_start(out=i0s[:], in_=i0s_in.ap())
            u0s = pone.tile([P, NC32, D], fp8, tag="u0s")
            nc.scalar.dma_start(out=u0s[:], in_=u0s_in.ap())
            ru_t, ri_t = [], []
            for b in range(nb):
                r1 = pdeg.tile([P, C4, 1], f32, tag="ru", name=f"ru{b}")
                nc.scalar.dma_start(out=r1[:], in_=ru_in[b].ap())
                r2 = pdeg.tile([P, C4, 1], f32, tag="ri", name=f"ri{b}")
                nc.scalar.dma_start(out=r2[:], in_=ri_in[b].ap())
                ru_t.append(r1)
                ri_t.append(r2)

            # bulk shard loads: all A's first (u-side), then all B's
            Atl, Btl = {}, {}
            H = NC32 // 2
            for b in range(nb):
                t = pshA.tile([P, NC32, ULOC], fp8, tag="A", name=f"A{b}")
                nc.sync.dma_start(out=t[:, 0:H, :], in_=A_in[b].ap()[:, 0:H, :])
                nc.sync.dma_start(out=t[:, H:NC32, :],
                                  in_=A_in[b].ap()[:, H:NC32, :])
                Atl[b] = t
            for b in range(nb):
                t = pshB.tile([P, NC32, ULOC], fp8, tag="B", name=f"B{b}")
                nc.sync.dma_start(out=t[:, 0:H, :], in_=B_in[b].ap()[:, 0:H, :])
                nc.sync.dma_start(out=t[:, H:NC32, :],
                                  in_=B_in[b].ap()[:, H:NC32, :])
                Btl[b] = t

            st_u, st_i = {}, {}

            def front_u(b):
                # P1: u1_raw.T = i0.T @ A   (DoubleRow, 16 MMs)
                PS1 = pps13.tile([D, ULOC], f32, tag="ps13", name=f"ps1_{b}")
                for g in range(NG):
                    nc.tensor.matmul(PS1[:], i0s[:, 2 * g:2 * g + 2, :],
                                     Atl[b][:, 2 * g:2 * g + 2, :],
                                     start=(g == 0), stop=(g == NG - 1),
                                     perf_mode=DR)
                S1 = pS1.tile([D, ULOC], f32, tag="S1", name=f"S1_{b}")
                nc.vector.tensor_copy(out=S1[:], in_=PS1[:])
                PT = ptr.tile([P, C4, D], f32, tag="pt", name=f"ptu_{b}")
                for c in range(C4):
                    nc.tensor.transpose(PT[:, c, :], S1[:, c * P:(c + 1) * P],
                                        ident[0:D, 0:D])
                Cu = psm.tile([P, C4, D], fp8, tag="Cu", name=f"Cu_{b}")
                for c in range(C4):
                    nc.vector.tensor_scalar_mul(out=Cu[:, c, :],
                                                in0=PT[:, c, :],
                                                scalar1=ru_t[b][:, c, :])
                cu_d = pdr.tile([P, C4, D], fp8, tag="cu", name=f"cu_{b}")
                nc.scalar.dma_start(out=cu_d[:], in_=Cu[:])
                gu_d = pdr.tile([NCORES * P, C4, D], fp8, tag="gu",
                                name=f"gu_{b}", addr_space="Shared")
                nc.gpsimd.collective_compute(
                    "AllGather", ALU.bypass, replica_groups=rg,
                    ins=[cu_d.opt()], outs=[gu_d.opt()])
                u1f = pfb.tile([P, NC32, D], fp8, tag="u1f", name=f"u1f_{b}")
                nc.scalar.dma_start(
                    out=u1f[:].rearrange("p (r c) d -> p r c d", r=NCORES),
                    in_=gu_d[:].rearrange("(r p) c d -> p r c d", p=P))
                st_u[b] = (S1, u1f)

            def front_i(b):
                S1, u1f = st_u.pop(b)
                # pack L = [u0 | u1q] as DoubleRow lhsT
                L = pL.tile([P, NC32, 2 * D], fp8, tag="L", name=f"L_{b}")
                nc.vector.tensor_copy(out=L[:, :, 0:D], in_=u0s[:])
                nc.vector.tensor_copy(out=L[:, :, D:2 * D], in_=u1f[:])
                # P2: [i1_raw | 64*i2_raw].T = L.T @ B
                PS2 = pps2.tile([P, ULOC], f32, tag="ps2", name=f"ps2_{b}")
                for g in range(NG):
                    nc.tensor.matmul(PS2[:], L[:, 2 * g:2 * g + 2, :],
                                     Btl[b][:, 2 * g:2 * g + 2, :],
                                     start=(g == 0), stop=(g == NG - 1),
                                     perf_mode=DR)
                S2 = psm.tile([D, ULOC], f32, tag="S2", name=f"S2_{b}")
                nc.vector.tensor_copy(out=S2[:], in_=PS2[0:D, :])
                PT = ptr.tile([P, C4, D], f32, tag="pt", name=f"pti_{b}")
                for c in range(C4):
                    nc.tensor.transpose(PT[:, c, :], S2[:, c * P:(c + 1) * P],
                                        ident[0:D, 0:D])
                Ci = psm.tile([P, C4, D], fp8, tag="Ci", name=f"Ci_{b}")
                for c in range(C4):
                    nc.vector.tensor_scalar_mul(out=Ci[:, c, :],
                                                in0=PT[:, c, :],
                                                scalar1=ri_t[b][:, c, :])
                # i-side output: i1_raw + i2_raw, transposed layout
                io = pout.tile([D, ULOC], f32, tag="io", name=f"io_{b}")
                nc.vector.scalar_tensor_tensor(
                    out=io[:], in0=PS2[D:2 * D, :], scalar=1.0 / SC,
                    in1=S2[:], op0=ALU.mult, op1=ALU.add)
                nc.scalar.dma_start(out=iT_out[b].ap(), in_=io[:])
                ci_d = pdr.tile([P, C4, D], fp8, tag="ci", name=f"ci_{b}")
                nc.scalar.dma_start(out=ci_d[:], in_=Ci[:])
                gi_d = pdr.tile([NCORES * P, C4, D], fp8, tag="gi",
                                name=f"gi_{b}", addr_space="Shared")
                nc.gpsimd.collective_compute(
                    "AllGather", ALU.bypass, replica_groups=rg,
                    ins=[ci_d.opt()], outs=[gi_d.opt()])
                i1f = pfb.tile([P, NC32, D], fp8, tag="i1f", name=f"i1f_{b}")
                nc.scalar.dma_start(
                    out=i1f[:].rearrange("p (r c) d -> p r c d", r=NCORES),
                    in_=gi_d[:].rearrange("(r p) c d -> p r c d", p=P))
                st_i[b] = (S1, i1f)

            def back_u(b):
                S1, i1f = st_i.pop(b)
                # P3: 64*u2_raw.T = i1q.T @ A
                PS3 = pps13.tile([D, ULOC], f32, tag="ps13", name=f"ps3_{b}")
                for g in range(NG):
                    nc.tensor.matmul(PS3[:], i1f[:, 2 * g:2 * g + 2, :],
                                     Atl[b][:, 2 * g:2 * g + 2, :],
                                     start=(g == 0), stop=(g == NG - 1),
                                     perf_mode=DR)
                uo = pout.tile([D, ULOC], f32, tag="uo", name=f"uo_{b}")
                nc.vector.scalar_tensor_tensor(
                    out=uo[:], in0=PS3[:], scalar=1.0 / SC,
                    in1=S1[:], op0=ALU.mult, op1=ALU.add)
                nc.scalar.dma_start(out=uT_out[b].ap(), in_=uo[:])

            for b in range(nb):
                front_u(b)
            fi = bu = 0
            while bu < nb:
                if fi < nb and fi - bu < 2:
                    front_i(fi)
                    fi += 1
                else:
                    back_u(bu)
                    bu += 1

    nc.compile()
    return nc


# --------------------------------------------------------------------------
# host-side helpers
# --------------------------------------------------------------------------

def _swz(x8):
    """[4096, D] fp8 -> [128, 32, D] with row = c*128 + p."""
    return np.ascontiguousarray(x8.reshape(NC32, P, x8.shape[1]).transpose(1, 0, 2))


def host_prep_behavior(R):
    """fp8 shards (both layouts, all cores) + degree reciprocals."""
    R = np.asarray(R, np.float32)
    R8 = R.astype(_FP8)
    # A (row-shard, transposed layout): A[k][p, ic, u] = R[k*512+u, ic*128+p]
    A_all = np.ascontiguousarray(
        R8.T.reshape(NC32, P, NCORES, ULOC).transpose(2, 1, 0, 3))
    # B (col-shard, natural layout): B[k][p, uc, i] = R[uc*128+p, k*512+i]
    B_all = np.ascontiguousarray(
        R8.reshape(NC32, P, NCORES, ULOC).transpose(2, 1, 0, 3))
    du = R.sum(axis=1, dtype=np.float64) + EPS     # [4096] user degrees
    di = R.sum(axis=0, dtype=np.float64) + EPS     # [4096] item degrees
    ru_all = np.ascontiguousarray(
        (SC / du).astype(np.float32).reshape(NCORES, C4, P)
        .transpose(0, 2, 1))[..., None]
    ri_all = np.ascontiguousarray(
        (SC / di).astype(np.float32).reshape(NCORES, C4, P)
        .transpose(0, 2, 1))[..., None]
    return A_all, B_all, ru_all, ri_all, du, di


def prep_in_maps(prepped, u0, i0):
    """prepped: list of host_prep_behavior outputs."""
    i0s = _swz(i0.astype(_FP8))
    u0s = _swz(u0.astype(_FP8))
    in_maps = []
    for k in range(NCORES):
        m = {"i0s": i0s, "u0s": u0s}
        for b, (A_all, B_all, ru_all, ri_all, _, _) in enumerate(prepped):
            m[f"A{b}"] = A_all[k]
            m[f"B{b}"] = B_all[k]
            m[f"ru{b}"] = np.ascontiguousarray(ru_all[k])
            m[f"ri{b}"] = np.ascontiguousarray(ri_all[k])
        in_maps.append(m)
    return in_maps


def assemble_dense(results, prepped):
    """Per-behavior (u_acc [N,D], i_acc [N,D]) from per-core outputs."""
    out = []
    for b, (_, _, _, _, du, di) in enumerate(prepped):
        su = (0.5 / du).astype(np.float32)
        si = (0.5 / di).astype(np.float32)
        u = np.empty((N, D), np.float32)
        it = np.empty((N, D), np.float32)
        for k in range(NCORES):
            sl = slice(k * ULOC, (k + 1) * ULOC)
            u[sl] = results[k][f"uT{b}"].T * su[sl][:, None]
            it[sl] = results[k][f"iT{b}"].T * si[sl][:, None]
        out.append((u, it))
    return out


def ones_behavior(u0, i0):
    """Analytic LightGCN-2-layer outputs when R is all-ones [N, N]."""
    s_i = i0.astype(np.float64).sum(axis=0)
    s_u = u0.astype(np.float64).sum(axis=0)
    d = N + EPS
    u_row = (s_i / d + s_u * N / (d * d)) * 0.5
    i_row = (s_u / d + s_i * N / (d * d)) * 0.5
    u = np.broadcast_to(u_row.astype(np.float32), (N, D)).copy()
    it = np.broadcast_to(i_row.astype(np.float32), (N, D)).copy()
    return u, it


# --------------------------------------------------------------------------
# cached device runner (compile once per behavior-count, run many)
# --------------------------------------------------------------------------

_RUNNERS = {}


class _Runner:
    def __init__(self, nb):
        self.nb = nb
        self.nc = build_program(nb)
        self._jitted = None
        self._meta = None

    def _prep_jit(self):
        import jax
        import numpy as _np
        from jax.sharding import Mesh, PartitionSpec
        from jax.experimental.shard_map import shard_map
        from concourse import bass2jax
        from concourse.bass2jax import _bass_exec_p, partition_id_tensor
        import concourse.mybir as mybir

        bass2jax.install_neuronx_cc_hook()
        nc = self.nc
        partition_name = (nc.partition_id_tensor.name
                          if nc.partition_id_tensor else None)
        in_names, out_names, out_avals, zero_shapes = [], [], [], []
        for alloc in nc.m.functions[0].allocations:
            if not isinstance(alloc, mybir.MemoryLocationSet):
                continue
            name = alloc.memorylocations[0].name
            if alloc.kind == "ExternalInput":
                if name != partition_name:
                    in_names.append(name)
            elif alloc.kind == "ExternalOutput":
                shape = tuple(alloc.tensor_shape)
                dtype = mybir.dt.np(alloc.dtype)
                out_names.append(name)
                out_avals.append(jax.core.ShapedArray(shape, dtype))
                zero_shapes.append((shape, dtype))
        n_params = len(in_names)
        full_in_names = list(in_names) + list(out_names)
        if partition_name is not None:
            full_in_names.append(partition_name)

        def _body(*args):
            operands = list(args)
            if partition_name is not None:
                operands.append(partition_id_tensor())
            outs = _bass_exec_p.bind(
                *operands,
                out_avals=tuple(out_avals),
                in_names=tuple(full_in_names),
                out_names=tuple(out_names),
                lowering_input_output_aliases=(),
                sim_require_finite=True,
                sim_require_nnan=True,
                nc=nc,
            )
            return tuple(outs)

        devices = jax.devices()[:NCORES]
        mesh = Mesh(_np.asarray(devices), ("core",))
        n_outs = len(out_names)
        in_specs = (PartitionSpec("core"),) * (n_params + n_outs)
        out_specs = (PartitionSpec("core"),) * n_outs
        donate = tuple(range(n_params, n_params + n_outs))
        self._jitted = jax.jit(
            shard_map(_body, mesh=mesh, in_specs=in_specs,
                      out_specs=out_specs, check_rep=False),
            donate_argnums=donate, keep_unused=True)
        self._meta = (in_names, out_names, out_avals, zero_shapes, n_params)

    def run(self, in_maps):
        if self._jitted is None:
            self._prep_jit()
        import numpy as _np
        in_names, out_names, out_avals, zero_shapes, n_params = self._meta
        concat_in = [
            _np.concatenate([_np.asarray(in_maps[c][nm]) for c in range(NCORES)],
                            axis=0)
            for nm in in_names]
        concat_zeros = [_np.zeros((NCORES * s[0], *s[1:]), dt)
                        for (s, dt) in zero_shapes]
        out_arrs = self._jitted(*concat_in, *concat_zeros)
        results = []
        for c in range(NCORES):
            results.append({
                nm: _np.asarray(out_arrs[i]).reshape(
                    NCORES, *out_avals[i].shape)[c]
                for i, nm in enumerate(out_names)})
        return results

    def run_traced(self, in_maps, tmpdir=None):
        """Run through run_bass_kernel_spmd with NTFF tracing (recompiles)."""
        _install_trace_shims()
        from concourse.bass_utils import run_bass_kernel_spmd
        return run_bass_kernel_spmd(self.nc, in_maps,
                                    core_ids=list(range(NCORES)),
                                    trace=True, tmpdir=tmpdir)


def _install_trace_shims():
    """This image's antenv lacks axon_hooks (the NTFF-hook registry) and has
    no artifact bucket; recreate the hook from the boot recipe and make
    artifact upload a local no-op."""
    import sys, types, importlib.util

    if "antenv.axon_hooks" not in sys.modules:
        mod = types.ModuleType("antenv.axon_hooks")
        mod._hook = None

        def set_axon_ntff_profile_hook(h):
            mod._hook = h

        def get_axon_ntff_profile_hook():
            return mod._hook

        mod.set_axon_ntff_profile_hook = set_axon_ntff_profile_hook
        mod.get_axon_ntff_profile_hook = get_axon_ntff_profile_hook
        import antenv
        sys.modules["antenv.axon_hooks"] = mod
        antenv.axon_hooks = mod

        spec = importlib.util.spec_from_file_location(
            "trn_boot_shim", "/root/.axon_site/trn_agent_boot/trn_boot.py")
        boot = importlib.util.module_from_spec(spec)
        spec.loader.exec_module(boot)
        hook = boot._ntff_profile_via_ctypes("/opt/axon/libaxon_pjrt.so")
        mod._hook = hook

    import concourse.bass_utils as bu
    if not getattr(bu.upload_artifacts, "_is_local_shim", False):
        def _local_upload(tmpdir):
            return tmpdir
        _local_upload._is_local_shim = True
        bu.upload_artifacts = _local_upload


def get_runner(nb):
    if nb not in _RUNNERS:
        _RUNNERS[nb] = _Runner(nb)
    return _RUNNERS[nb]


# --------------------------------------------------------------------------
# entry point
# --------------------------------------------------------------------------

def _is_ones(a):
    return a[0, 0] == 1.0 and bool(np.all(a == np.float32(1.0)))


def kernel(**inputs):
    inputs = {k: np.asarray(v) for k, v in inputs.items()}
    u0 = np.ascontiguousarray(inputs["user_embedding"], dtype=np.float32)
    i0 = np.ascontiguousarray(inputs["item_embedding"], dtype=np.float32)

    real_names = ["R_click", "R_fav", "R_cart", "R_buy"]
    virt_names = [("M_click", "add_click"), ("M_fav", "add_fav"),
                  ("M_cart", "add_cart")]
    mats = [np.asarray(inputs[n], dtype=np.float32) for n in real_names]
    mats += [np.asarray(inputs[m], dtype=np.float32) for m, _ in virt_names]

    dense_idx = [j for j, a in enumerate(mats) if not _is_ones(a)]
    per_behavior = [None] * 7

    if dense_idx:
        nb = len(dense_idx)
        runner = get_runner(nb)
        prepped = [host_prep_behavior(mats[j]) for j in dense_idx]
        in_maps = prep_in_maps(prepped, u0, i0)
        results = runner.run(in_maps)
        dense = assemble_dense(results, prepped)
        for pos, j in enumerate(dense_idx):
            per_behavior[j] = dense[pos]

    ones_cache = None
    for j, a in enumerate(mats):
        if per_behavior[j] is None:
            if ones_cache is None:
                ones_cache = ones_behavior(u0, i0)
            per_behavior[j] = ones_cache

    ur = [per_behavior[j][0] for j in range(4)]
    ir = [per_behavior[j][1] for j in range(4)]
    uv = [per_behavior[4 + j][0] + np.asarray(inputs[virt_names[j][1]],
                                              dtype=np.float32)
          for j in range(3)]
    iv = [per_behavior[4 + j][1] for j in range(3)]

    out = np.concatenate(
        [np.stack(ur), np.stack(ir), np.stack(uv), np.stack(iv)], axis=0)
    return np.ascontiguousarray(out, dtype=np.float32)
